# revision 1
# baseline (speedup 1.0000x reference)
"""Causal attention (B=4, S=4096, D=768) on 8 Trainium2 NeuronCores.

Sharding: zigzag query-strip packing. Each batch b is handled by two cores
(roles): role 0 owns query strips {0,2,5,7}, role 1 owns {1,3,4,6} (strips of
512 rows). Both roles run the IDENTICAL program (SPMD): 4 query supers of 512
rows, with per-super key-block loop bounds TSLOT=(8,16,24,32) 128-row blocks.
Strip->slot assignment is chosen so each role's strip needs <= the slot bound;
the overshoot plus the causal diagonal are killed by additive mask planes
(host-precomputed per role, supplied as input data). Softmax uses no
max-subtraction (scores/sqrt(D) ~ N(0,1); exp is safe in fp32); the
denominator comes free from a ones-column appended to V. Host prep: cast to
bf16, transpose x, pack query strips (layout-only work; all FLOPs on device).
"""

import math

import numpy as np
import ml_dtypes

P = 128
NEG = -1e9
bf16 = ml_dtypes.bfloat16

# Full-size problem geometry (hardcoded; kernel.py must be self-contained).
B, S, D = 4, 4096, 768
SUP = 512
NSLOT = 4
NQ = NSLOT * SUP
TSLOT = (8, 16, 24, 32)
MASK_KB = 8
ROLE_STRIPS = ((0, 2, 5, 7), (1, 3, 4, 6))
N_CORES = 8


def build_program(S, D, SUP, TSLOT, MASK_KB, out_dtype_np=np.float32):
    """Build the single SPMD Bass program (one core's view).

    Inputs (per core): xkT bf16 [D,S], xqT bf16 [D,NQ], wq/wk/wv bf16 [D,D],
    rmask f32 [NSLOT, P, MASK_KB*SUP]. Output: out f32 [NQ, D] (packed rows).
    """
    import concourse.bass as bass
    import concourse.tile as tile
    import concourse.mybir as mybir
    from concourse import bacc

    DC = D // P
    NSLOT_ = len(TSLOT)
    NQ_ = NSLOT_ * SUP
    NKB = S // P
    ED = D + 1  # V gets a ones column appended -> denominator for free
    # free-dim splits for the PV matmul over the augmented [0, ED) columns
    osplits = []
    pos = 0
    while pos < ED:
        osplits.append((pos, min(pos + 512, ED)))
        pos = min(pos + 512, ED)
    # splits of [0, D) for the V projection
    vsplits = []
    pos = 0
    while pos < D:
        vsplits.append((pos, min(pos + 512, D)))
        pos = min(pos + 512, D)
    SCALE = 1.0 / math.sqrt(float(D))
    f32 = mybir.dt.float32
    b16 = mybir.dt.bfloat16

    nc = bacc.Bacc("TRN2", target_bir_lowering=False, debug=False)

    xkT = nc.dram_tensor("xkT", [D, S], b16, kind="ExternalInput").ap()
    xqT = nc.dram_tensor("xqT", [D, NQ_], b16, kind="ExternalInput").ap()
    whs = {
        n: nc.dram_tensor(n, [D, D], b16, kind="ExternalInput").ap()
        for n in ("wq", "wk", "wv")
    }
    rmask = nc.dram_tensor(
        "rmask", [NSLOT_, P, MASK_KB * SUP], f32, kind="ExternalInput"
    ).ap()
    out = nc.dram_tensor(
        "out", [NQ_, D], mybir.dt.from_np(np.dtype(out_dtype_np)), kind="ExternalOutput"
    ).ap()

    xkT_r = xkT.rearrange("(c p) s -> p c s", p=P)
    xqT_r = xqT.rearrange("(c p) s -> p c s", p=P)

    with tile.TileContext(nc) as tc:
        with (
            tc.tile_pool(name="persist", bufs=1) as persist,
            tc.tile_pool(name="xstage", bufs=3) as xstage,
        ):
            # persistent SBUF tensors
            KT = persist.tile([P, DC, S], b16, name="KT")        # K^T, d on partitions
            QT = persist.tile([P, DC, NQ_], b16, name="QT")      # Q^T
            V = persist.tile([P, NKB, ED], b16, name="V")       # V by key-block, +ones col
            nc.vector.memset(V[:, :, D:ED], 1.0)

            # ---------------- phase 1: projections ----------------
            with (
                tc.tile_pool(name="wpool", bufs=1) as wpool,
                tc.tile_pool(name="ppsum", bufs=4, space="PSUM") as ppsum,
            ):
                W = {}
                for n in ("wq", "wk", "wv"):
                    W[n] = wpool.tile([P, DC, D], b16, tag=n, name=n)
                    nc.sync.dma_start(W[n], whs[n].rearrange("(c p) e -> p c e", p=P))

                def project_chunk(xT_t, s0, width, kt_dst, with_v):
                    """xT_t: [P, DC, width] bf16 chunk of x^T starting at col s0."""
                    for do in range(DC):
                        ps = ppsum.tile([P, 512], f32, tag="proj", name="proj_ps")[:, :width]
                        wsrc = W["wk"] if kt_dst is KT else W["wq"]
                        for dc in range(DC):
                            nc.tensor.matmul(
                                ps,
                                lhsT=wsrc[:, dc, do * P:(do + 1) * P],
                                rhs=xT_t[:, dc, :width],
                                start=(dc == 0),
                                stop=(dc == DC - 1),
                            )
                        nc.any.tensor_copy(
                            out=kt_dst[:, do, s0:s0 + width], in_=ps
                        )
                    if with_v:
                        for sb in range(width // P):
                            kb = (s0 + sb * P) // P
                            for (e0, e1) in vsplits:
                                ps = ppsum.tile([P, 512], f32, tag="projv", name="projv_ps")[:, :e1 - e0]
                                for dc in range(DC):
                                    nc.tensor.matmul(
                                        ps,
                                        lhsT=xT_t[:, dc, sb * P:(sb + 1) * P],
                                        rhs=W["wv"][:, dc, e0:e1],
                                        start=(dc == 0),
                                        stop=(dc == DC - 1),
                                    )
                                nc.any.tensor_copy(
                                    out=V[:, kb, e0:e1], in_=ps
                                )

                CHUNK = 512
                for ch in range(S // CHUNK):
                    xT_t = xstage.tile([P, DC, CHUNK], b16, tag="xk", name="xk_t")
                    nc.sync.dma_start(
                        xT_t, xkT_r[:, :, ch * CHUNK:(ch + 1) * CHUNK]
                    )
                    project_chunk(xT_t, ch * CHUNK, CHUNK, KT, with_v=True)
                for ch in range(NQ_ // CHUNK):
                    xT_t = xstage.tile([P, DC, CHUNK], b16, tag="xq", name="xq_t")
                    nc.sync.dma_start(
                        xT_t, xqT_r[:, :, ch * CHUNK:(ch + 1) * CHUNK]
                    )
                    project_chunk(xT_t, ch * CHUNK, CHUNK, QT, with_v=False)

            # ---------------- phase 2: attention ----------------
            with (
                tc.tile_pool(name="expp", bufs=1) as expp,
                tc.tile_pool(name="mpool", bufs=2) as mpool,
                tc.tile_pool(name="opool", bufs=2) as opool,
                tc.tile_pool(name="spsum", bufs=2, space="PSUM") as spsum,
                tc.tile_pool(name="opsumA", bufs=2, space="PSUM") as opsumA,
                tc.tile_pool(name="opsumB", bufs=2, space="PSUM") as opsumB,
            ):
                Tmax = max(TSLOT)
                expT = expp.tile([P, Tmax, SUP], b16, name="expT")
                for t in range(NSLOT_):
                    T = TSLOT[t]
                    q0 = t * SUP
                    # scores + exp for all key blocks of this super
                    for kb in range(T):
                        ps = spsum.tile([P, SUP], f32, tag="sc", name="sc_ps")
                        for dc in range(DC):
                            nc.tensor.matmul(
                                ps,
                                lhsT=KT[:, dc, kb * P:(kb + 1) * P],
                                rhs=QT[:, dc, q0:q0 + SUP],
                                start=(dc == 0),
                                stop=(dc == DC - 1),
                            )
                        if kb >= T - MASK_KB:
                            kbi = kb - (T - MASK_KB)
                            m = mpool.tile([P, SUP], f32, tag="m", name="m_t")
                            nc.sync.dma_start(
                                m, rmask[t, :, kbi * SUP:(kbi + 1) * SUP]
                            )
                            nc.vector.tensor_add(ps, ps, m)
                        nc.scalar.activation(
                            expT[:, kb, :], ps,
                            mybir.ActivationFunctionType.Exp, scale=SCALE,
                        )
                    # out = (expT)^T @ [V | 1] per 128-row query slice
                    for sl in range(SUP // P):
                        bound = min(T, T - (SUP // P) + 1 + sl)
                        pss = []
                        for (e0, e1) in osplits:
                            pss.append(
                                opsumA.tile([P, 512], f32, tag="oA", name="oA_ps")[:, :e1 - e0]
                                if e0 == 0
                                else opsumB.tile([P, ED - 512], f32, tag="oB", name="oB_ps")
                            )
                        for kb in range(bound):
                            for (e0, e1), ps_o in zip(osplits, pss):
                                nc.tensor.matmul(
                                    ps_o,
                                    lhsT=expT[:, kb, sl * P:(sl + 1) * P],
                                    rhs=V[:, kb, e0:e1],
                                    start=(kb == 0),
                                    stop=(kb == bound - 1),
                                )
                        recip = opool.tile([P, 1], f32, tag="recip", name="recip_t")
                        last = pss[-1]
                        nc.vector.reciprocal(recip, last[:, last.shape[-1] - 1:])
                        ot = opool.tile([P, D], mybir.dt.from_np(np.dtype(out_dtype_np)), tag="ot", name="ot_t")
                        for (e0, e1), ps_o in zip(osplits, pss):
                            hi = min(e1, D)
                            nc.vector.tensor_scalar_mul(
                                ot[:, e0:hi], ps_o[:, :hi - e0], recip
                            )
                        nc.sync.dma_start(
                            out[q0 + sl * P: q0 + (sl + 1) * P, :], ot
                        )

    nc.compile()
    return nc


def make_rmask(role_strips, TSLOT, SUP, MASK_KB):
    nslot = len(TSLOT)
    m = np.zeros((nslot, P, MASK_KB * SUP), np.float32)
    i = np.arange(P)[:, None]
    j = np.arange(SUP)[None, :]
    for t in range(nslot):
        q0 = SUP * role_strips[t]
        T = TSLOT[t]
        for kbi in range(MASK_KB):
            k0 = P * (T - MASK_KB + kbi)
            m[t, :, kbi * SUP:(kbi + 1) * SUP] = np.where(
                q0 + j >= k0 + i, 0.0, NEG
            )
    return m


_nc_cache = {}
last_run = None


def _get_nc():
    key = (S, D, SUP, TSLOT, MASK_KB)
    if key not in _nc_cache:
        _nc_cache[key] = build_program(S, D, SUP, TSLOT, MASK_KB)
    return _nc_cache[key]



def make_in_maps(x, w_b):
    rmasks = [make_rmask(ROLE_STRIPS[r], TSLOT, SUP, MASK_KB) for r in range(2)]
    in_maps = []
    for c in range(N_CORES):
        b, role = c % B, c // B
        xb = x[b].astype(bf16)
        xq = np.concatenate(
            [xb[SUP * s:SUP * (s + 1)] for s in ROLE_STRIPS[role]], axis=0
        )
        in_maps.append({
            "xkT": np.ascontiguousarray(xb.T),
            "xqT": np.ascontiguousarray(xq.T),
            "rmask": rmasks[role],
            **w_b,
        })
    return in_maps


def kernel(x, Wq, Wk, Wv):
    from concourse import bass_utils

    x = np.asarray(x, dtype=np.float32)
    w_b = {n: np.asarray(w, np.float32).astype(bf16)
           for n, w in (("wq", Wq), ("wk", Wk), ("wv", Wv))}

    nc = _get_nc()

    in_maps = make_in_maps(x, w_b)

    global last_run
    last_run = bass_utils.run_bass_kernel_spmd(
        nc, in_maps, core_ids=list(range(N_CORES))
    )
    res = last_run.results

    out = np.empty((B, S, D), np.float32)
    for c in range(N_CORES):
        b, role = c % B, c // B
        packed = res[c]["out"]
        for t, s in enumerate(ROLE_STRIPS[role]):
            out[b, SUP * s:SUP * (s + 1)] = packed[SUP * t:SUP * (t + 1)]
    return out


if __name__ == "__main__":
    import reference

    inputs = {k: np.asarray(v) for k, v in reference.setup_inputs().items()}
    expected = np.asarray(reference.reference(**inputs))
    actual = kernel(**inputs)
    err = np.abs(actual - expected).max()
    print(f"absmax err: {err:.3e}  rel: {err / np.abs(expected).max():.3e}")

